# revision 28
# baseline (speedup 1.0000x reference)
"""MultiHeadAttention forward on 8 TRN2 NeuronCores.

Sharding: core c -> (batch b = c//2, query-half qh = c%2). Each core computes
the full attention output for 1024 query rows of one batch element (all 16
heads); outputs are disjoint slices, no collective needed.

All matmuls run in bf16 (1 col/cycle like f32r, but lower power: avoids the
HAM k=4/8 throttle that capped the f32r version, and halves SBUF/DMA).
Host-side folds: 0.125 scale into wq/bq; bv into bo (bo' = bo + wo@bv, exact
because softmax weights sum to 1); k-sequence order is permuted per core so
the core's own query block is always columns 0-1023 of inT (keeps the SPMD
program identical across cores with no separate q input).

Everything stays in SBUF (qT 2MB, kT 4MB, V 2.1MB, ctx 2MB bf16) - no DRAM
spills. Phase A (projections) is partially interleaved into phase B's PE
queue as deadline-scheduled fillers so the PE never idles while the Scalar
engine works through the 282us of exp().

Per-core math (transposed activation layout, dim on partitions):
  qT = wqT.T @ inT[:, 0:1024] + bq     [1024, 1024] bf16 SBUF
  kT = wkT.T @ inT              + bk   [1024, 2048] bf16 SBUF
  V  = inT.T @ wvT                     [2048, 16*(64|ones)] bf16 SBUF
  per head h: sT = kT_h.T @ qT_h       [2048, 1024] strips of [128, 1024]
              e  = exp(sT + maskbias)  (ACT, bf16 out)
              ctxT_aug = V_aug_h.T @ e [65, 1024]; row 64 = softmax denom
              ctxT = ctxT_aug[0:64] * recip(bcast(denom))
  out = ctxT_all.T @ woT + bo'         [1024, 1024] f32
"""

import numpy as np
import ml_dtypes


def _round_f32r(x):
    """RNE to 11 mantissa bits - matches the PE's fp32->fp32r rounding."""
    b = np.ascontiguousarray(x, dtype=np.float32).view(np.uint32).astype(np.uint64)
    lsb = (b >> 12) & 1
    b = (b + 0x7FF + lsb) & 0xFFFFF000
    return b.astype(np.uint32).view(np.float32)

import concourse.bacc as bacc
import concourse.tile as tile
import concourse.mybir as mybir
from concourse.bass_utils import run_bass_kernel_spmd

F32 = mybir.dt.float32
F32R = mybir.dt.float32r
BF16 = mybir.dt.bfloat16
EXP = mybir.ActivationFunctionType.Exp
BFNP = ml_dtypes.bfloat16

BS, QLEN, DIM, H, DPH = 4, 2048, 1024, 16, 64
NC_ = 8
LQ = 1024  # local query rows per core

_PROG = None


def _build():
    nc = bacc.Bacc("TRN2", target_bir_lowering=False, debug=False, num_devices=NC_)

    INT = nc.dram_tensor("inT", [DIM, QLEN], BF16, kind="ExternalInput").ap()
    # wq/wk/wv are host-pre-arranged chunk-major ([chunk, partition, it*cols])
    # so every weight DMA is a fully contiguous 2KB-per-partition line load
    WQT = nc.dram_tensor("wqT", [8, 128, 1024], BF16, kind="ExternalInput").ap()
    WKT = nc.dram_tensor("wkT", [8, 128, 1024], BF16, kind="ExternalInput").ap()
    WVT = nc.dram_tensor("wvT", [2, 128, 4096], BF16, kind="ExternalInput").ap()
    WOT = nc.dram_tensor("woT", [DIM, DIM], BF16, kind="ExternalInput").ap()
    BQC = nc.dram_tensor("bqc", [DIM, 1], F32, kind="ExternalInput").ap()
    BKC = nc.dram_tensor("bkc", [DIM, 1], F32, kind="ExternalInput").ap()
    BOR = nc.dram_tensor("boR", [1, DIM], F32R, kind="ExternalInput").ap()
    MBC = nc.dram_tensor("mb", [QLEN, 1], F32, kind="ExternalInput").ap()
    OUT = nc.dram_tensor("out", [LQ, DIM], F32, kind="ExternalOutput").ap()

    with tile.TileContext(nc) as tc:
        from contextlib import ExitStack
        with ExitStack() as ctx:
            const_p = ctx.enter_context(tc.tile_pool(name="const", bufs=1))
            big_p = ctx.enter_context(tc.tile_pool(name="big", bufs=1))
            wpool = ctx.enter_context(tc.tile_pool(name="wts", bufs=2))
            stage_p = ctx.enter_context(tc.tile_pool(name="stage", bufs=2))
            bex = ctx.enter_context(tc.tile_pool(name="bex", bufs=4))
            misc_p = ctx.enter_context(tc.tile_pool(name="misc", bufs=2))
            out_p = ctx.enter_context(tc.tile_pool(name="outp", bufs=2))
            psS = ctx.enter_context(tc.tile_pool(name="psS", bufs=2, space="PSUM"))
            psC = ctx.enter_context(tc.tile_pool(name="psC", bufs=1, space="PSUM"))
            psA = ctx.enter_context(tc.tile_pool(name="psA", bufs=2, space="PSUM"))

            # ---- constants ----
            ones_f = const_p.tile([1, 128], F32, tag="onesf")
            nc.vector.memset(ones_f[:], 1.0)
            ones1r = const_p.tile([1, 128], F32R, tag="ones1r")
            nc.vector.tensor_copy(ones1r[:], ones_f[:])
            ones16 = const_p.tile([128, 16], F32, tag="ones16")
            nc.vector.memset(ones16[:], 1.0)
            bq_t = const_p.tile([128, 8], F32, tag="bq")
            nc.sync.dma_start(bq_t[:], BQC.rearrange("(g p) o -> p (g o)", p=128))
            bk_t = const_p.tile([128, 8], F32, tag="bk")
            nc.sync.dma_start(bk_t[:], BKC.rearrange("(g p) o -> p (g o)", p=128))
            mb_t = const_p.tile([128, 16], F32, tag="mb")
            nc.sync.dma_start(mb_t[:], MBC.rearrange("(g p) o -> p (g o)", p=128))
            bo_sb = const_p.tile([1, DIM], F32R, tag="bo")
            nc.sync.dma_start(bo_sb[:], BOR[:])

            # ---- persistent SBUF tensors ----
            inT = big_p.tile([128, 8, QLEN], BF16, tag="inT", name="inT_sb")
            qT = [big_p.tile([128, LQ], BF16, tag=f"q{hp}", name=f"qT{hp}")
                  for hp in range(8)]
            kT = [big_p.tile([128, QLEN], BF16, tag=f"k{hp}", name=f"kT{hp}")
                  for hp in range(8)]
            V_sb = [big_p.tile([128, H * 65], BF16, tag=f"v{st}", name=f"V{st}")
                    for st in range(16)]
            ctx_all = [big_p.tile([128, LQ], BF16, tag=f"c{dt}", name=f"ctx{dt}")
                       for dt in range(8)]
            bobc = big_p.tile([128, DIM], F32, tag="bobc", name="bobc")

            # early weight prefetch + input chunks, split across two engine
            # queues: sync = [consts, wq0, it odds], gpsimd = [it evens,
            # wk0]. First q matmul gates on wq0+it0 (~12us), the q chain
            # completes when the last chunk lands (~18us).
            wq0_t = wpool.tile([128, 8, 128], BF16, tag="wq")
            nc.sync.dma_start(
                wq0_t[:], WQT[0].rearrange("p (a b) -> p a b", a=8))
            INT_re = INT.rearrange("(it p) m -> p it m", p=128)
            for it in range(0, 8, 2):
                nc.gpsimd.dma_start(inT[:, it, :], INT_re[:, it, :])
            for it in range(1, 8, 2):
                nc.sync.dma_start(inT[:, it, :], INT_re[:, it, :])
            wk0_t = wpool.tile([128, 8, 128], BF16, tag="wk")
            nc.gpsimd.dma_start(
                wk0_t[:], WKT[0].rearrange("p (a b) -> p a b", a=8))

            def emit_bobc():
                # broadcast bo' to 128 partitions (runs as a late filler)
                for oc in range(2):
                    pboc = psA.tile([128, 512], F32, tag="a", name="pboc")
                    nc.tensor.matmul(pboc[:], ones1r[:, 0:128],
                                     bo_sb[:, oc * 512:(oc + 1) * 512],
                                     start=True, stop=True)
                    nc.vector.tensor_copy(bobc[:, oc * 512:(oc + 1) * 512],
                                          pboc[:])

            # ---- phase A emitters (each returns PE work ~8 matmuls) ----
            def emit_q(hp, oc):
                if oc == 0:
                    if hp == 0:
                        emit_q.wq = wq0_t
                    else:
                        wq_t = wpool.tile([128, 8, 128], BF16, tag="wq")
                        nc.sync.dma_start(
                            wq_t[:], WQT[hp].rearrange("p (a b) -> p a b", a=8))
                        emit_q.wq = wq_t
                wq_t = emit_q.wq
                ps = psA.tile([128, 512], F32, tag="a", name="psq")
                for it in range(8):
                    nc.tensor.matmul(ps[:], wq_t[:, it, :],
                                     inT[:, it, oc * 512:(oc + 1) * 512],
                                     start=(it == 0), stop=(it == 7))
                nc.vector.tensor_scalar_add(
                    qT[hp][:, oc * 512:(oc + 1) * 512], ps[:], bq_t[:, hp:hp + 1])

            def emit_k(hp, sc):
                if sc == 0:
                    if hp == 0:
                        emit_k.wk = wk0_t
                    else:
                        wk_t = wpool.tile([128, 8, 128], BF16, tag="wk")
                        nc.sync.dma_start(
                            wk_t[:], WKT[hp].rearrange("p (a b) -> p a b", a=8))
                        emit_k.wk = wk_t
                wk_t = emit_k.wk
                ps = psA.tile([128, 512], F32, tag="a", name="psk")
                for it in range(8):
                    nc.tensor.matmul(ps[:], wk_t[:, it, :],
                                     inT[:, it, sc * 512:(sc + 1) * 512],
                                     start=(it == 0), stop=(it == 7))
                nc.vector.tensor_scalar_add(
                    kT[hp][:, sc * 512:(sc + 1) * 512], ps[:], bk_t[:, hp:hp + 1])

            def emit_v_dma(oc):
                wv_t = wpool.tile([128, 8, 512], BF16, tag="wv", bufs=1)
                nc.sync.dma_start(
                    wv_t[:], WVT[oc].rearrange("p (a b) -> p a b", a=8))
                emit_v_dma.wv = wv_t

            def emit_v(oc, st):
                wv_t = emit_v_dma.wv
                ps = psA.tile([128, 512], F32, tag="a", name="psv")
                for it in range(8):
                    nc.tensor.matmul(ps[:], inT[:, it, st * 128:(st + 1) * 128],
                                     wv_t[:, it, :], start=(it == 0), stop=(it == 7))
                dst = V_sb[st][:].rearrange("p (h c) -> p h c", c=65)
                nc.vector.tensor_copy(
                    dst[:, oc * 8:(oc + 1) * 8, 0:64],
                    ps[:].rearrange("p (h c) -> p h c", c=64))
                if oc == 0:
                    # ones cols for ALL heads (written once, before any head runs)
                    nc.vector.tensor_copy(V_sb[st][:, 64::65], ones16[:])

            wo_t = [None] * 8

            def emit_wo_dma(dt):
                t = out_p.tile([128, DIM], BF16, tag=f"wo{dt}", name=f"wo{dt}",
                               bufs=1)
                nc.sync.dma_start(t[:], WOT[dt * 128:(dt + 1) * 128, :])
                wo_t[dt] = t

            # ---- filler schedule ----
            # units run inside phase B wherever the PE would otherwise stall.
            # deadline = (head, strip) by which the unit must have run.
            fillers = []

            def add(head, strip, fn):
                fillers.append((head * 16 + strip, fn))

            # uniform spread positions for the A remainder, clamped to need-by.
            # q/k for hp must complete before stage(2hp), issued at
            # slot (2hp-1)*16+8 -- use that as the hard deadline.
            units = []
            for hp in range(1, 8):
                stage_slot = (2 * hp - 1) * 16 + 8
                for oc in range(2):
                    units.append((lambda hp=hp, oc=oc: emit_q(hp, oc),
                                  stage_slot))
                for sc in range(4):
                    units.append((lambda hp=hp, sc=sc: emit_k(hp, sc),
                                  stage_slot))
                if hp == 4:
                    units.append((lambda: emit_v_dma(1), 8 * 16 - 40))
                    for st in range(16):
                        units.append((lambda st=st: emit_v(1, st),
                                      8 * 16 + st - 2))
            n_units = len(units)
            span_slots = 15 * 16  # spread over heads 0..14
            for i, (fn, need_by) in enumerate(units):
                slot = min(int(i * span_slots / n_units), need_by - 4)
                fillers.append((max(slot, 0), fn))
            for dt in range(8):
                fillers.append((14 * 16 + dt * 2, lambda dt=dt: emit_wo_dma(dt)))
            fillers.append((14 * 16 + 4, emit_bobc))
            fillers.sort(key=lambda x: x[0])
            fidx = [0]

            def pump(slot):
                while fidx[0] < len(fillers) and fillers[fidx[0]][0] <= slot:
                    fillers[fidx[0]][1]()
                    fidx[0] += 1

            # ---- phase A-pre: minimum to start head 0 ----
            for oc in range(2):
                emit_q(0, oc)
            for sc in range(4):
                emit_k(0, sc)
            emit_v_dma(0)
            for st in range(16):
                emit_v(0, st)

            # ---- per-head staging: copy the head's 64-partition q/k slices
            # to offset-0 tiles (PE operands at partition offset 64 unproven)
            def stage(h):
                hp, half = h // 2, h % 2
                qt = stage_p.tile([64, LQ], BF16, tag="qt", name="qt")
                nc.gpsimd.dma_start(qt[:], qT[hp][half * 64:(half + 1) * 64, :])
                kt_sb = stage_p.tile([64, QLEN], BF16, tag="kt", name="kt")
                nc.gpsimd.dma_start(kt_sb[:], kT[hp][half * 64:(half + 1) * 64, :])
                return qt, kt_sb

            staged = stage(0)

            # ---- phase B: attention per head, fillers pumped in ----
            # ctx is computed in NATURAL layout (q on partitions, 65 cols =
            # 64 dph + denominator) - uses all 128 output partitions, so the
            # ctx matmuls cost 65 cycles per q-strip instead of 512 per
            # q-half. Normalization becomes a per-partition tensor_scalar.
            # A matmul's PSUM output cannot cross a 2KB bank, so the 8
            # q-strips are split across two [128, 4, 65] accumulators.
            for h in range(H):
                hp, half = h // 2, h % 2
                qt, kt_sb = staged
                cn = [psC.tile([128, 4, 65], F32, tag="cn", name="cn", bufs=2)
                      for _ in range(2)]
                for kt in range(16):
                    ps_s = psS.tile([128, LQ], F32, tag="s", name="s")
                    for qc in range(2):
                        nc.tensor.matmul(ps_s[:, qc * 512:(qc + 1) * 512],
                                         kt_sb[:, kt * 128:(kt + 1) * 128],
                                         qt[:, qc * 512:(qc + 1) * 512],
                                         start=True, stop=True)
                    pump(h * 16 + kt)
                    if kt == 8 and h < H - 1:
                        staged = stage(h + 1)
                    ex = bex.tile([128, LQ], BF16, tag="ex", name="ex")
                    nc.scalar.activation(ex[:], ps_s[:], EXP,
                                         bias=mb_t[:, kt:kt + 1])
                    # one accumulation group per PSUM bank: start=True clears
                    # has_written for the WHOLE bank, so only the first
                    # region may set it; later regions' first write lands on
                    # cleared bits and overwrites, then accumulates.
                    for qs in range(8):
                        nc.tensor.matmul(cn[qs // 4][:, qs % 4, :],
                                         ex[:, qs * 128:(qs + 1) * 128],
                                         V_sb[kt][:, h * 65:(h + 1) * 65],
                                         start=(kt == 0 and qs % 4 == 0),
                                         stop=(kt == 15 and qs % 4 == 3))
                # normalize per q row: recip of col 64, scale cols 0-63
                dcol = misc_p.tile([128, 8], F32, tag="dc", name="dcol")
                for t in range(2):
                    nc.vector.tensor_copy(dcol[:, t * 4:(t + 1) * 4],
                                          cn[t][:, :, 64])
                rcol = misc_p.tile([128, 8], F32, tag="rc", name="rcol")
                nc.vector.reciprocal_approx_fast(rcol[:], dcol[:])
                ctxn = misc_p.tile([128, 8, DPH], BF16, tag="cx", name="ctxn")
                for qs in range(8):
                    nc.vector.tensor_scalar_mul(
                        ctxn[:, qs, :], cn[qs // 4][:, qs % 4, 0:64],
                        rcol[:, qs:qs + 1])
                # transpose [q, dph] -> [dph, q] via DMA xbar (2 q-strips at
                # a time), then shift into the head's ctx_all partitions
                for j in range(4):
                    tp = misc_p.tile([128, 128], BF16, tag="tp", name="tp",
                                     bufs=4)
                    nc.sync.dma_start(
                        tp[:], ctxn[:, 2 * j:2 * j + 2, :].rearrange(
                            "p a b -> p (a b)"), transpose=True)
                    for i in range(2):
                        nc.vector.tensor_copy(
                            ctx_all[hp][half * 64:(half + 1) * 64,
                                        (2 * j + i) * 128:(2 * j + i + 1) * 128],
                            tp[i * 64:(i + 1) * 64, :])

            pump(16 * 16)  # drain any remaining fillers (wo DMAs)

            # ---- phase C: output projection ----
            for st in range(8):
                for oc in range(2):
                    po = psA.tile([128, 512], F32, tag="a", name="po")
                    for dt in range(8):
                        nc.tensor.matmul(po[:],
                                         ctx_all[dt][:, st * 128:(st + 1) * 128],
                                         wo_t[dt][:, oc * 512:(oc + 1) * 512],
                                         start=(dt == 0), stop=(dt == 7))
                    ot = out_p.tile([128, 512], F32, tag="ot", name="ot")
                    nc.vector.tensor_add(ot[:], po[:],
                                         bobc[:, oc * 512:(oc + 1) * 512])
                    nc.sync.dma_start(
                        OUT[st * 128:(st + 1) * 128, oc * 512:(oc + 1) * 512],
                        ot[:])

    nc.compile()
    return nc


def _get_prog():
    global _PROG
    if _PROG is None:
        _PROG = _build()
    return _PROG


def kernel(input, mask, wq, bq, wk, bk, wv, bv, wo, bo, _trace=False):
    nc = _get_prog()

    input = np.asarray(input, np.float32)
    mask = np.asarray(mask)
    wq, bq = np.asarray(wq, np.float32), np.asarray(bq, np.float32)
    wk, bk = np.asarray(wk, np.float32), np.asarray(bk, np.float32)
    wv, bv = np.asarray(wv, np.float32), np.asarray(bv, np.float32)
    wo, bo = np.asarray(wo, np.float32), np.asarray(bo, np.float32)

    def _chunk_major(wT, n_chunks):
        # [dim_in, dim_out] -> [chunk, partition, it*cols]: on-chip layout
        # [p, it, cols] per out-chunk, stored contiguous for fast DMA
        cols = DIM // n_chunks
        a = wT.reshape(8, 128, n_chunks, cols)          # [it, p, chunk, m]
        return np.ascontiguousarray(
            a.transpose(2, 1, 0, 3).reshape(n_chunks, 128, 8 * cols))

    wqT = _chunk_major((wq.T * 0.125).astype(BFNP), 8)
    wkT = _chunk_major(wk.T.astype(BFNP), 8)
    wvT = _chunk_major(wv.T.astype(BFNP), 2)
    woT = np.ascontiguousarray(wo.T.astype(BFNP))
    bqc = (bq * 0.125).reshape(DIM, 1).astype(np.float32)
    bkc = bk.reshape(DIM, 1)
    # bv folded into bo: softmax weights sum to 1 exactly by construction
    boR = _round_f32r(
        (bo.astype(np.float64) + wo.astype(np.float64) @ bv.astype(np.float64)
         ).astype(np.float32)).reshape(1, DIM)
    mbias = [np.where(mask[b] == 0, np.float32(-30.0), np.float32(0.0))
             .astype(np.float32) for b in range(BS)]
    inT_bf = [np.ascontiguousarray(input[b].T).astype(BFNP) for b in range(BS)]

    in_maps = []
    for c in range(NC_):
        b, qh = c // 2, c % 2
        if qh == 0:
            inT_c = inT_bf[b]
            mb_c = mbias[b]
        else:
            # permute k-sequence so our q block is always columns 0-1023;
            # attention output is invariant to k order when mask follows
            inT_c = np.ascontiguousarray(
                np.concatenate([inT_bf[b][:, LQ:], inT_bf[b][:, :LQ]], axis=1))
            mb_c = np.concatenate([mbias[b][LQ:], mbias[b][:LQ]])
        in_maps.append({
            "inT": inT_c,
            "wqT": wqT, "wkT": wkT, "wvT": wvT, "woT": woT,
            "bqc": bqc, "bkc": bkc, "boR": boR,
            "mb": mb_c.reshape(QLEN, 1),
        })

    res = run_bass_kernel_spmd(nc, in_maps, list(range(NC_)), trace=_trace)

    out = np.empty((BS, QLEN, DIM), np.float32)
    for c in range(NC_):
        b, qh = c // 2, c % 2
        out[b, qh * LQ:(qh + 1) * LQ, :] = res.results[c]["out"]
    if _trace:
        kernel.last_exec_time_ns = res.exec_time_ns
        kernel.last_results = res
    return out


# revision 29
# speedup vs baseline: 1.1908x; 1.1908x over previous
"""MultiHeadAttention forward on 8 TRN2 NeuronCores.

Sharding: core c -> (batch b = c//2, query-half qh = c%2). Each core computes
the full attention output for 1024 query rows of one batch element (all 16
heads); outputs are disjoint slices, no collective needed.

All matmuls run in bf16 (1 col/cycle like f32r, but lower power: avoids the
HAM k=4/8 throttle that capped the f32r version, and halves SBUF/DMA).
Host-side folds: 0.125 scale into wq/bq; bv into bo (bo' = bo + wo@bv, exact
because softmax weights sum to 1); k-sequence order is permuted per core so
the core's own query block is always columns 0-1023 of inT (keeps the SPMD
program identical across cores with no separate q input).

Everything stays in SBUF (qT 2MB, kT 4MB, V 2.1MB, ctx 2MB bf16) - no DRAM
spills. Phase A (projections) is partially interleaved into phase B's PE
queue as deadline-scheduled fillers so the PE never idles while the Scalar
engine works through the 282us of exp().

Per-core math (transposed activation layout, dim on partitions):
  qT = wqT.T @ inT[:, 0:1024] + bq     [1024, 1024] bf16 SBUF
  kT = wkT.T @ inT              + bk   [1024, 2048] bf16 SBUF
  V  = inT.T @ wvT                     [2048, 16*(64|ones)] bf16 SBUF
  per head h: sT = kT_h.T @ qT_h       [2048, 1024] strips of [128, 1024]
              e  = exp(sT + maskbias)  (ACT, bf16 out)
              ctxT_aug = V_aug_h.T @ e [65, 1024]; row 64 = softmax denom
              ctxT = ctxT_aug[0:64] * recip(bcast(denom))
  out = ctxT_all.T @ woT + bo'         [1024, 1024] f32
"""

import numpy as np
import ml_dtypes


def _round_f32r(x):
    """RNE to 11 mantissa bits - matches the PE's fp32->fp32r rounding."""
    b = np.ascontiguousarray(x, dtype=np.float32).view(np.uint32).astype(np.uint64)
    lsb = (b >> 12) & 1
    b = (b + 0x7FF + lsb) & 0xFFFFF000
    return b.astype(np.uint32).view(np.float32)

import concourse.bacc as bacc
import concourse.tile as tile
import concourse.mybir as mybir
from concourse.bass_utils import run_bass_kernel_spmd

F32 = mybir.dt.float32
F32R = mybir.dt.float32r
BF16 = mybir.dt.bfloat16
EXP = mybir.ActivationFunctionType.Exp
BFNP = ml_dtypes.bfloat16

BS, QLEN, DIM, H, DPH = 4, 2048, 1024, 16, 64
NC_ = 8
LQ = 1024  # local query rows per core

_PROG = None


def _build():
    nc = bacc.Bacc("TRN2", target_bir_lowering=False, debug=False, num_devices=NC_)

    INT = nc.dram_tensor("inT", [DIM, QLEN], BF16, kind="ExternalInput").ap()
    WQT = nc.dram_tensor("wqT", [DIM, DIM], BF16, kind="ExternalInput").ap()
    WKT = nc.dram_tensor("wkT", [DIM, DIM], BF16, kind="ExternalInput").ap()
    WVT = nc.dram_tensor("wvT", [DIM, DIM], BF16, kind="ExternalInput").ap()
    WOT = nc.dram_tensor("woT", [DIM, DIM], BF16, kind="ExternalInput").ap()
    BQC = nc.dram_tensor("bqc", [DIM, 1], F32, kind="ExternalInput").ap()
    BKC = nc.dram_tensor("bkc", [DIM, 1], F32, kind="ExternalInput").ap()
    BOR = nc.dram_tensor("boR", [1, DIM], F32R, kind="ExternalInput").ap()
    MBC = nc.dram_tensor("mb", [QLEN, 1], F32, kind="ExternalInput").ap()
    OUT = nc.dram_tensor("out", [LQ, DIM], F32, kind="ExternalOutput").ap()

    with tile.TileContext(nc) as tc:
        from contextlib import ExitStack
        with ExitStack() as ctx:
            const_p = ctx.enter_context(tc.tile_pool(name="const", bufs=1))
            big_p = ctx.enter_context(tc.tile_pool(name="big", bufs=1))
            wpool = ctx.enter_context(tc.tile_pool(name="wts", bufs=2))
            stage_p = ctx.enter_context(tc.tile_pool(name="stage", bufs=2))
            bex = ctx.enter_context(tc.tile_pool(name="bex", bufs=3))
            misc_p = ctx.enter_context(tc.tile_pool(name="misc", bufs=2))
            out_p = ctx.enter_context(tc.tile_pool(name="outp", bufs=2))
            psS = ctx.enter_context(tc.tile_pool(name="psS", bufs=2, space="PSUM"))
            psC = ctx.enter_context(tc.tile_pool(name="psC", bufs=1, space="PSUM"))
            psA = ctx.enter_context(tc.tile_pool(name="psA", bufs=2, space="PSUM"))

            # ---- early weight prefetch: q0/k0 gate the first matmuls ----
            wq0_t = wpool.tile([128, 8, 128], BF16, tag="wq")
            nc.sync.dma_start(
                wq0_t[:], WQT[:, 0:128].rearrange("(it p) m -> p it m", p=128))
            wk0_t = wpool.tile([128, 8, 128], BF16, tag="wk")
            nc.sync.dma_start(
                wk0_t[:], WKT[:, 0:128].rearrange("(it p) m -> p it m", p=128))

            # ---- constants ----
            ones_f = const_p.tile([1, 128], F32, tag="onesf")
            nc.vector.memset(ones_f[:], 1.0)
            ones1r = const_p.tile([1, 128], F32R, tag="ones1r")
            nc.vector.tensor_copy(ones1r[:], ones_f[:])
            ones16 = const_p.tile([128, 16], F32, tag="ones16")
            nc.vector.memset(ones16[:], 1.0)
            bq_t = const_p.tile([128, 8], F32, tag="bq")
            nc.sync.dma_start(bq_t[:], BQC.rearrange("(g p) o -> p (g o)", p=128))
            bk_t = const_p.tile([128, 8], F32, tag="bk")
            nc.sync.dma_start(bk_t[:], BKC.rearrange("(g p) o -> p (g o)", p=128))
            mb_t = const_p.tile([128, 16], F32, tag="mb")
            nc.sync.dma_start(mb_t[:], MBC.rearrange("(g p) o -> p (g o)", p=128))
            bo_sb = const_p.tile([1, DIM], F32R, tag="bo")
            nc.sync.dma_start(bo_sb[:], BOR[:])

            # ---- persistent SBUF tensors ----
            inT = big_p.tile([128, 8, QLEN], BF16, tag="inT", name="inT_sb")
            qT = [big_p.tile([128, LQ], BF16, tag=f"q{hp}", name=f"qT{hp}")
                  for hp in range(8)]
            kT = [big_p.tile([128, QLEN], BF16, tag=f"k{hp}", name=f"kT{hp}")
                  for hp in range(8)]
            V_sb = [big_p.tile([128, H * 65], BF16, tag=f"v{st}", name=f"V{st}")
                    for st in range(16)]
            ctx_all = [big_p.tile([128, LQ], BF16, tag=f"c{dt}", name=f"ctx{dt}")
                       for dt in range(8)]
            bobc = big_p.tile([128, DIM], F32, tag="bobc", name="bobc")

            # input: chunked DMAs split across two engine queues so the
            # first matmuls don't wait for the full 4MB on one ring. Weight
            # chunks for q0/k0 are issued first (they're small and gate the
            # first matmul).
            INT_re = INT.rearrange("(it p) m -> p it m", p=128)
            for it in range(8):
                eng = nc.gpsimd if it % 2 else nc.sync
                eng.dma_start(inT[:, it, :], INT_re[:, it, :])

            # broadcast bo' to 128 partitions (off critical path)
            for oc in range(2):
                pboc = psA.tile([128, 512], F32, tag="a", name="pboc")
                nc.tensor.matmul(pboc[:], ones1r[:, 0:128],
                                 bo_sb[:, oc * 512:(oc + 1) * 512],
                                 start=True, stop=True)
                nc.vector.tensor_copy(bobc[:, oc * 512:(oc + 1) * 512], pboc[:])

            # ---- phase A emitters (each returns PE work ~8 matmuls) ----
            def emit_q(hp, oc):
                if oc == 0:
                    if hp == 0:
                        emit_q.wq = wq0_t
                    else:
                        wq_t = wpool.tile([128, 8, 128], BF16, tag="wq")
                        nc.sync.dma_start(
                            wq_t[:],
                            WQT[:, hp * 128:(hp + 1) * 128].rearrange(
                                "(it p) m -> p it m", p=128))
                        emit_q.wq = wq_t
                wq_t = emit_q.wq
                ps = psA.tile([128, 512], F32, tag="a", name="psq")
                for it in range(8):
                    nc.tensor.matmul(ps[:], wq_t[:, it, :],
                                     inT[:, it, oc * 512:(oc + 1) * 512],
                                     start=(it == 0), stop=(it == 7))
                nc.vector.tensor_scalar_add(
                    qT[hp][:, oc * 512:(oc + 1) * 512], ps[:], bq_t[:, hp:hp + 1])

            def emit_k(hp, sc):
                if sc == 0:
                    if hp == 0:
                        emit_k.wk = wk0_t
                    else:
                        wk_t = wpool.tile([128, 8, 128], BF16, tag="wk")
                        nc.sync.dma_start(
                            wk_t[:],
                            WKT[:, hp * 128:(hp + 1) * 128].rearrange(
                                "(it p) m -> p it m", p=128))
                        emit_k.wk = wk_t
                wk_t = emit_k.wk
                ps = psA.tile([128, 512], F32, tag="a", name="psk")
                for it in range(8):
                    nc.tensor.matmul(ps[:], wk_t[:, it, :],
                                     inT[:, it, sc * 512:(sc + 1) * 512],
                                     start=(it == 0), stop=(it == 7))
                nc.vector.tensor_scalar_add(
                    kT[hp][:, sc * 512:(sc + 1) * 512], ps[:], bk_t[:, hp:hp + 1])

            def emit_v_dma(oc):
                wv_t = wpool.tile([128, 8, 512], BF16, tag="wv", bufs=1)
                nc.sync.dma_start(
                    wv_t[:],
                    WVT[:, oc * 512:(oc + 1) * 512].rearrange(
                        "(it p) m -> p it m", p=128))
                emit_v_dma.wv = wv_t

            def emit_v(oc, st):
                wv_t = emit_v_dma.wv
                ps = psA.tile([128, 512], F32, tag="a", name="psv")
                for it in range(8):
                    nc.tensor.matmul(ps[:], inT[:, it, st * 128:(st + 1) * 128],
                                     wv_t[:, it, :], start=(it == 0), stop=(it == 7))
                dst = V_sb[st][:].rearrange("p (h c) -> p h c", c=65)
                nc.vector.tensor_copy(
                    dst[:, oc * 8:(oc + 1) * 8, 0:64],
                    ps[:].rearrange("p (h c) -> p h c", c=64))
                if oc == 0:
                    # ones cols for ALL heads (written once, before any head runs)
                    nc.vector.tensor_copy(V_sb[st][:, 64::65], ones16[:])

            wo_t = [None] * 8

            def emit_wo_dma(dt):
                t = out_p.tile([128, DIM], BF16, tag=f"wo{dt}", name=f"wo{dt}",
                               bufs=1)
                nc.sync.dma_start(t[:], WOT[dt * 128:(dt + 1) * 128, :])
                wo_t[dt] = t

            # ---- filler schedule ----
            # units run inside phase B wherever the PE would otherwise stall.
            # deadline = (head, strip) by which the unit must have run.
            fillers = []

            def add(head, strip, fn):
                fillers.append((head * 16 + strip, fn))

            # uniform spread positions for the A remainder, clamped to need-by.
            # q/k for hp must complete before stage(2hp), issued at
            # slot (2hp-1)*16+8 -- use that as the hard deadline.
            units = []
            for hp in range(1, 8):
                stage_slot = (2 * hp - 1) * 16 + 8
                for oc in range(2):
                    units.append((lambda hp=hp, oc=oc: emit_q(hp, oc),
                                  stage_slot))
                for sc in range(4):
                    units.append((lambda hp=hp, sc=sc: emit_k(hp, sc),
                                  stage_slot))
                if hp == 4:
                    units.append((lambda: emit_v_dma(1), 8 * 16 - 40))
                    for st in range(16):
                        units.append((lambda st=st: emit_v(1, st),
                                      8 * 16 + st - 2))
            n_units = len(units)
            span_slots = 15 * 16  # spread over heads 0..14
            for i, (fn, need_by) in enumerate(units):
                slot = min(int(i * span_slots / n_units), need_by - 4)
                fillers.append((max(slot, 0), fn))
            for dt in range(8):
                fillers.append((14 * 16 + dt * 2, lambda dt=dt: emit_wo_dma(dt)))
            fillers.sort(key=lambda x: x[0])
            fidx = [0]

            def pump(slot):
                while fidx[0] < len(fillers) and fillers[fidx[0]][0] <= slot:
                    fillers[fidx[0]][1]()
                    fidx[0] += 1

            # ---- phase A-pre: minimum to start head 0 ----
            for oc in range(2):
                emit_q(0, oc)
            for sc in range(4):
                emit_k(0, sc)
            emit_v_dma(0)
            for st in range(16):
                emit_v(0, st)

            # ---- per-head staging: copy the head's 64-partition q/k slices
            # to offset-0 tiles (PE operands at partition offset 64 unproven)
            def stage(h):
                hp, half = h // 2, h % 2
                qt = stage_p.tile([64, LQ], BF16, tag="qt", name="qt")
                nc.gpsimd.dma_start(qt[:], qT[hp][half * 64:(half + 1) * 64, :])
                kt_sb = stage_p.tile([64, QLEN], BF16, tag="kt", name="kt")
                nc.gpsimd.dma_start(kt_sb[:], kT[hp][half * 64:(half + 1) * 64, :])
                return qt, kt_sb

            staged = stage(0)

            # ---- phase B: attention per head, fillers pumped in ----
            # ctx is computed in NATURAL layout (q on partitions, 65 cols =
            # 64 dph + denominator) - uses all 128 output partitions, so the
            # ctx matmuls cost 65 cycles per q-strip instead of 512 per
            # q-half. Normalization becomes a per-partition tensor_scalar.
            # A matmul's PSUM output cannot cross a 2KB bank, so the 8
            # q-strips are split across two [128, 4, 65] accumulators.
            for h in range(H):
                hp, half = h // 2, h % 2
                qt, kt_sb = staged
                cn = [psC.tile([128, 4, 65], F32, tag="cn", name="cn", bufs=2)
                      for _ in range(2)]
                for kt in range(16):
                    ps_s = psS.tile([128, LQ], F32, tag="s", name="s")
                    for qc in range(2):
                        nc.tensor.matmul(ps_s[:, qc * 512:(qc + 1) * 512],
                                         kt_sb[:, kt * 128:(kt + 1) * 128],
                                         qt[:, qc * 512:(qc + 1) * 512],
                                         start=True, stop=True)
                    pump(h * 16 + kt)
                    if kt == 8 and h < H - 1:
                        staged = stage(h + 1)
                    ex = bex.tile([128, LQ], BF16, tag="ex", name="ex")
                    nc.scalar.activation(ex[:], ps_s[:], EXP,
                                         bias=mb_t[:, kt:kt + 1])
                    # one accumulation group per PSUM bank: start=True clears
                    # has_written for the WHOLE bank, so only the first
                    # region may set it; later regions' first write lands on
                    # cleared bits and overwrites, then accumulates.
                    for qs in range(8):
                        nc.tensor.matmul(cn[qs // 4][:, qs % 4, :],
                                         ex[:, qs * 128:(qs + 1) * 128],
                                         V_sb[kt][:, h * 65:(h + 1) * 65],
                                         start=(kt == 0 and qs % 4 == 0),
                                         stop=(kt == 15 and qs % 4 == 3))
                # normalize per q row: recip of col 64, scale cols 0-63
                dcol = misc_p.tile([128, 8], F32, tag="dc", name="dcol")
                for t in range(2):
                    nc.vector.tensor_copy(dcol[:, t * 4:(t + 1) * 4],
                                          cn[t][:, :, 64])
                rcol = misc_p.tile([128, 8], F32, tag="rc", name="rcol")
                nc.vector.reciprocal_approx_fast(rcol[:], dcol[:])
                ctxn = misc_p.tile([128, 8, DPH], BF16, tag="cx", name="ctxn")
                for qs in range(8):
                    nc.vector.tensor_scalar_mul(
                        ctxn[:, qs, :], cn[qs // 4][:, qs % 4, 0:64],
                        rcol[:, qs:qs + 1])
                # transpose [q, dph] -> [dph, q] via DMA xbar (2 q-strips at
                # a time), then shift into the head's ctx_all partitions
                for j in range(4):
                    tp = misc_p.tile([128, 128], BF16, tag="tp", name="tp",
                                     bufs=4)
                    nc.sync.dma_start(
                        tp[:], ctxn[:, 2 * j:2 * j + 2, :].rearrange(
                            "p a b -> p (a b)"), transpose=True)
                    for i in range(2):
                        nc.vector.tensor_copy(
                            ctx_all[hp][half * 64:(half + 1) * 64,
                                        (2 * j + i) * 128:(2 * j + i + 1) * 128],
                            tp[i * 64:(i + 1) * 64, :])

            pump(16 * 16)  # drain any remaining fillers (wo DMAs)

            # ---- phase C: output projection ----
            for st in range(8):
                for oc in range(2):
                    po = psA.tile([128, 512], F32, tag="a", name="po")
                    for dt in range(8):
                        nc.tensor.matmul(po[:],
                                         ctx_all[dt][:, st * 128:(st + 1) * 128],
                                         wo_t[dt][:, oc * 512:(oc + 1) * 512],
                                         start=(dt == 0), stop=(dt == 7))
                    ot = out_p.tile([128, 512], F32, tag="ot", name="ot")
                    nc.vector.tensor_add(ot[:], po[:],
                                         bobc[:, oc * 512:(oc + 1) * 512])
                    nc.sync.dma_start(
                        OUT[st * 128:(st + 1) * 128, oc * 512:(oc + 1) * 512],
                        ot[:])

    nc.compile()
    return nc


def _get_prog():
    global _PROG
    if _PROG is None:
        _PROG = _build()
    return _PROG


def kernel(input, mask, wq, bq, wk, bk, wv, bv, wo, bo, _trace=False):
    nc = _get_prog()

    input = np.asarray(input, np.float32)
    mask = np.asarray(mask)
    wq, bq = np.asarray(wq, np.float32), np.asarray(bq, np.float32)
    wk, bk = np.asarray(wk, np.float32), np.asarray(bk, np.float32)
    wv, bv = np.asarray(wv, np.float32), np.asarray(bv, np.float32)
    wo, bo = np.asarray(wo, np.float32), np.asarray(bo, np.float32)

    wqT = np.ascontiguousarray((wq.T * 0.125).astype(BFNP))
    wkT = np.ascontiguousarray(wk.T.astype(BFNP))
    wvT = np.ascontiguousarray(wv.T.astype(BFNP))
    woT = np.ascontiguousarray(wo.T.astype(BFNP))
    bqc = (bq * 0.125).reshape(DIM, 1).astype(np.float32)
    bkc = bk.reshape(DIM, 1)
    # bv folded into bo: softmax weights sum to 1 exactly by construction
    boR = _round_f32r(
        (bo.astype(np.float64) + wo.astype(np.float64) @ bv.astype(np.float64)
         ).astype(np.float32)).reshape(1, DIM)
    mbias = [np.where(mask[b] == 0, np.float32(-30.0), np.float32(0.0))
             .astype(np.float32) for b in range(BS)]
    inT_bf = [np.ascontiguousarray(input[b].T).astype(BFNP) for b in range(BS)]

    in_maps = []
    for c in range(NC_):
        b, qh = c // 2, c % 2
        if qh == 0:
            inT_c = inT_bf[b]
            mb_c = mbias[b]
        else:
            # permute k-sequence so our q block is always columns 0-1023;
            # attention output is invariant to k order when mask follows
            inT_c = np.ascontiguousarray(
                np.concatenate([inT_bf[b][:, LQ:], inT_bf[b][:, :LQ]], axis=1))
            mb_c = np.concatenate([mbias[b][LQ:], mbias[b][:LQ]])
        in_maps.append({
            "inT": inT_c,
            "wqT": wqT, "wkT": wkT, "wvT": wvT, "woT": woT,
            "bqc": bqc, "bkc": bkc, "boR": boR,
            "mb": mb_c.reshape(QLEN, 1),
        })

    res = run_bass_kernel_spmd(nc, in_maps, list(range(NC_)), trace=_trace)

    out = np.empty((BS, QLEN, DIM), np.float32)
    for c in range(NC_):
        b, qh = c // 2, c % 2
        out[b, qh * LQ:(qh + 1) * LQ, :] = res.results[c]["out"]
    if _trace:
        kernel.last_exec_time_ns = res.exec_time_ns
        kernel.last_results = res
    return out
